# revision 41
# baseline (speedup 1.0000x reference)
"""Causal self-attention on 8 trn2 NeuronCores.

Problem: B=4, T=2048, D=1024, H=16 heads (Dh=64), fp32 in/out, causal
softmax attention with 4 linear projections (biases are zero in this
problem's setup and are folded out).

Sharding (SPMD, one NEFF on all 8 cores, no collectives):
  core c -> batch b = c//2, head-group m = c%2 (8 heads each).
  Each core computes Q/K/V for its 8 heads over ALL 2048 tokens of its
  batch, runs causal attention, and produces a PARTIAL output
  projection (contracting only its 512 y-dims of Wo).  The host sums
  the two partial outputs per batch (Megatron row-parallel unshard).
  This halves the K/V projection work vs. a query-split sharding (no
  duplicated K/V) and halves the per-core output projection.

Device pipeline per core (all matmul inputs bf16, fp32 PSUM):
  - Q/K projections per head-pair into transposed layout qT/kT [d, t]
    (so scores S^T = kT.T @ qT need no on-device transposes).
  - V projection per key-block into natural layout [t, d] for all 8
    heads at once, stored bf16 with an appended ones column (the ones
    accumulate the softmax denominator during the exp(S) @ V matmul).
  - Attention runs per 512-column query chunk qc (kb <= 4*qc+3 by
    causality): both heads of a pair land in one 2-bank PSUM tile, one
    exp op (ScalarE, 1/8 scale folded in) covers both heads, a single
    128x128 triangular mask multiplies the diagonal block on GpSimd,
    exp(S) @ V accumulates per-chunk Y tiles [65, 512] in PSUM.
  - Per-pair normalize with no DRAM round trip: the two denominator
    rows are staged to SBUF, two K=1 PE matmuls broadcast them across
    the pair's 128 partitions into one PSUM bank, one DVE reciprocal
    + multiply normalizes the pair's y chunk in place.
  - The partial output projection + DMA-out run per query chunk,
    interleaved into the next chunk's attention as PE filler; its
    per-ct matmuls are data-gated on each pair's normalize, so the
    kernel tail is only the last pair's normalize + 8 tail matmuls.
  - Host-side input layouts are arranged so every big DMA reads
    contiguous >=2KB per-partition segments.
"""

import numpy as np
import ml_dtypes

import concourse.bass as bass
import concourse.mybir as mybir
import concourse.tile as tile
from concourse import bacc
from concourse.bass_utils import run_bass_kernel_spmd

B, T, D, H, DH = 4, 2048, 1024, 16, 64
P = 128
CT = D // P          # 8 contraction tiles over the model dim
NKB = T // P         # 16 key blocks
NQC = T // 512       # 4 query chunks of 512
NPAIR = 4            # local head pairs per core (8 heads)
NCORES = 8

f32 = mybir.dt.float32
bf16 = mybir.dt.bfloat16
AF = mybir.ActivationFunctionType


def build_kernel():
    nc = bacc.Bacc("TRN2", target_bir_lowering=False, debug=False)
    xT_d = nc.dram_tensor("xT", [NQC, P, CT, 512], bf16, kind="ExternalInput")
    wq_d = nc.dram_tensor("wq", [NPAIR, P, CT, P], bf16, kind="ExternalInput")
    wk_d = nc.dram_tensor("wk", [NPAIR, P, CT, P], bf16, kind="ExternalInput")
    wv_d = nc.dram_tensor("wv", [P, CT, 512], bf16, kind="ExternalInput")
    wo_d = nc.dram_tensor("wo", [P, NPAIR, D], bf16, kind="ExternalInput")
    mask_d = nc.dram_tensor("mask", [P, P], bf16, kind="ExternalInput")
    osel_d = nc.dram_tensor("osel", [P, 2, P], bf16, kind="ExternalInput")
    out_d = nc.dram_tensor("out", [T, D], bf16, kind="ExternalOutput")
    with tile.TileContext(nc) as tc:
        _emit(tc, xT_d, wq_d, wk_d, wv_d, wo_d, mask_d, osel_d, out_d)
    nc.compile()
    return nc


def _emit(tc, xT_d, wq_d, wk_d, wv_d, wo_d, mask_d, osel_d, out_d):
    nc = tc.nc
    with (
        tc.tile_pool(name="xt", bufs=1) as xt_pool,
        tc.tile_pool(name="w", bufs=1) as w_pool,
        tc.tile_pool(name="qkv", bufs=1) as qkv_pool,
        tc.tile_pool(name="ynorm", bufs=1) as ynorm_pool,
        tc.tile_pool(name="exp", bufs=6) as exp_pool,
        tc.tile_pool(name="den", bufs=2) as den_pool,
        tc.tile_pool(name="norm", bufs=2) as norm_pool,
        tc.tile_pool(name="osb", bufs=2) as osb_pool,
        tc.tile_pool(name="osbl", bufs=1) as osbl_pool,
        tc.tile_pool(name="ps_s", bufs=2, space="PSUM") as ps_s,
        tc.tile_pool(name="ps_y", bufs=2, space="PSUM") as ps_y,
        tc.tile_pool(name="ps_p", bufs=2, space="PSUM") as ps_p,
    ):
        xt = xt_pool.tile([P, NQC, CT, 512], bf16, name="xt")
        wq_sb = w_pool.tile([P, NPAIR, CT, P], bf16, name="wq")
        wk_sb = w_pool.tile([P, NPAIR, CT, P], bf16, name="wk")
        wv_sb = w_pool.tile([P, CT, 512], bf16, name="wv")
        wo_sb = w_pool.tile([P, NPAIR, D], bf16, name="wo")
        mask_sb = w_pool.tile([P, P], bf16, name="mask")
        osel_sb = w_pool.tile([P, 2, P], bf16, name="osel")
        qT = qkv_pool.tile([P, NPAIR, T], bf16, name="qT")
        kT = qkv_pool.tile([P, NPAIR, T], bf16, name="kT")
        v8 = qkv_pool.tile([P, NKB, 8, DH + 1], bf16, name="v8")
        ynorm = ynorm_pool.tile([P, NPAIR, T], bf16, name="ynorm")
        nc.vector.memset(v8[:, :, :, DH:DH + 1], 1.0)

        # Input DMAs, ordered so the first Q/K chains (pair 0, token
        # chunk 0) can start DMA-paced as early as possible.  All
        # sources are host-pre-arranged to be contiguous per partition;
        # the first chunk's x / wv transfers are split by ct-half so
        # the projection chains stream behind the DMA.
        HC = CT // 2
        nc.sync.dma_start(wq_sb[:, 0, :, :], wq_d.ap()[0])
        nc.sync.dma_start(xt[:, 0, 0:HC, :], xT_d.ap()[0][:, 0:HC, :])
        nc.sync.dma_start(wk_sb[:, 0, :, :], wk_d.ap()[0])
        nc.sync.dma_start(xt[:, 0, HC:CT, :], xT_d.ap()[0][:, HC:CT, :])
        nc.sync.dma_start(wv_sb[:, 0:HC, :], wv_d.ap()[:, 0:HC, :])
        nc.sync.dma_start(wv_sb[:, HC:CT, :], wv_d.ap()[:, HC:CT, :])
        for p in range(1, NPAIR):
            nc.sync.dma_start(wq_sb[:, p, :, :], wq_d.ap()[p])
            nc.sync.dma_start(wk_sb[:, p, :, :], wk_d.ap()[p])
        nc.sync.dma_start(mask_sb[:], mask_d.ap())
        nc.sync.dma_start(osel_sb[:], osel_d.ap())
        for tcc in range(1, NQC):
            nc.sync.dma_start(xt[:, tcc, :, :], xT_d.ap()[tcc])
        nc.sync.dma_start(wo_sb[:], wo_d.ap())

        def qk_chain(which, p, tcc):
            w_sb, dstT = (wq_sb, qT) if which == "q" else (wk_sb, kT)
            ps = ps_p.tile([P, 512], f32, tag="pp", name="ps_qk")
            for ct in range(CT):
                nc.tensor.matmul(
                    ps[:],
                    w_sb[:, p, ct, :],
                    xt[:, tcc, ct, :],
                    start=(ct == 0),
                    stop=(ct == CT - 1),
                )
            nc.vector.tensor_copy(
                dstT[:, p, tcc * 512:(tcc + 1) * 512], ps[:])

        def v_chain(kb):
            ps = ps_p.tile([P, 512], f32, tag="pp", name="ps_v")
            for ct in range(CT):
                nc.tensor.matmul(
                    ps[:],
                    xt[:, kb // 4, ct, (kb % 4) * P:(kb % 4 + 1) * P],
                    wv_sb[:, ct, :],
                    start=(ct == 0),
                    stop=(ct == CT - 1),
                )
            nc.vector.tensor_copy(
                v8[:, kb, :, 0:DH], ps[:].rearrange("p (h d) -> p h d", h=8)
            )

        fillers = []

        def tick():
            if fillers:
                fillers.pop(0)()

        def attn(p, qc):
            last = 4 * qc + 3
            sl = slice(qc * 512, (qc + 1) * 512)
            ys = [
                ps_y.tile([DH + 1, 512], f32, tag="y", name=f"y{hh}")
                for hh in range(2)
            ]
            for kb in range(last + 1):
                diag = kb >= 4 * qc
                s0 = kb * P if diag else qc * 512
                w = (qc + 1) * 512 - s0
                sc = ps_s.tile([P, 2, 512], f32, tag="s", name="sc")
                for hh in range(2):
                    nc.tensor.matmul(
                        sc[:, hh, 0:w],
                        kT[hh * DH:(hh + 1) * DH, p, kb * P:(kb + 1) * P],
                        qT[hh * DH:(hh + 1) * DH, p, s0:s0 + w],
                        start=True,
                        stop=True,
                    )
                expS = exp_pool.tile([P, 2, 512], bf16, tag="e", name="expS")
                nc.scalar.activation(
                    expS[:, :, 0:w], sc[:, :, 0:w], AF.Exp, scale=0.125
                )
                if diag:
                    for hh in range(2):
                        nc.vector.tensor_mul(
                            expS[:, hh, 0:P], expS[:, hh, 0:P], mask_sb[:]
                        )
                for hh in range(2):
                    nc.tensor.matmul(
                        ys[hh][:, s0 - qc * 512:s0 - qc * 512 + w],
                        v8[:, kb, 2 * p + hh, :],
                        expS[:, hh, 0:w],
                        start=(kb == 0),
                        stop=(kb == last),
                    )
                tick()
            # Drain: stage denominator rows and the pair's y chunk; the
            # normalize (two K=1 broadcast matmuls + reciprocal +
            # multiply) is deferred into the filler stream so the PE
            # never waits on the staging copies.
            dstg = den_pool.tile([DH + 1, 2, 512], bf16, tag="dstg",
                                 name="dstg")
            with nc.allow_low_precision(reason="bf16 denominator staging"):
                for hh in range(2):
                    nc.vector.tensor_copy(dstg[DH:DH + 1, hh, :],
                                          ys[hh][DH:DH + 1, :])
            for hh in range(2):
                nc.vector.tensor_copy(
                    ynorm[hh * DH:(hh + 1) * DH, p, sl], ys[hh][0:DH, :])

            def norm_pair(p=p, sl=sl, dstg=dstg):
                bc_ps = ps_p.tile([P, 512], f32, tag="pp", name="bc_ps")
                for hh in range(2):
                    nc.tensor.matmul(
                        bc_ps[:],
                        osel_sb[DH:DH + 1, hh, :],
                        dstg[DH:DH + 1, hh, :],
                        start=(hh == 0),
                        stop=(hh == 1),
                    )
                rec = norm_pool.tile([P, 512], f32, tag="rec", name="rec")
                nc.vector.reciprocal_approx_fast(rec[:], bc_ps[:])
                rec_h = norm_pool.tile([P, 512], bf16, tag="rech",
                                       name="rec_h")
                with nc.allow_low_precision(reason="bf16 recip"):
                    nc.vector.tensor_copy(rec_h[:], rec[:])
                nc.vector.tensor_mul(ynorm[:, p, sl], ynorm[:, p, sl],
                                     rec_h[:])

            fillers.append(norm_pair)

        def oproj_tb(qc, tb):
            col = qc * 512 + tb * P
            osb = osb_pool.tile([P, D], bf16, tag="osb", name="osb")
            for mh in range(2):
                ops = ps_p.tile([P, 512], f32, tag="pp", name="ops")
                for ct in range(NPAIR):
                    nc.tensor.matmul(
                        ops[:],
                        ynorm[:, ct, col:col + P],
                        wo_sb[:, ct, mh * 512:(mh + 1) * 512],
                        start=(ct == 0),
                        stop=(ct == NPAIR - 1),
                    )
                with nc.allow_low_precision(reason="bf16 partial out"):
                    nc.vector.tensor_copy(osb[:, mh * 512:(mh + 1) * 512],
                                          ops[:])
            nc.sync.dma_start(out_d.ap()[col:col + P, :], osb[:])

        # Tail-split output projection for the last chunk: the ct<3
        # partial contraction runs as filler during the last pair's
        # attention; only the 8 ct=3 matmuls + adds trail the final
        # normalize.
        last_osb = {}

        def oproj_a(tb, mh):
            col = (NQC - 1) * 512 + tb * P
            if tb not in last_osb:
                last_osb[tb] = osbl_pool.tile([P, D], bf16, name=f"osbl{tb}")
            osb = last_osb[tb]
            ops = ps_p.tile([P, 512], f32, tag="pp", name="ops_a")
            for ct in range(NPAIR - 1):
                nc.tensor.matmul(
                    ops[:],
                    ynorm[:, ct, col:col + P],
                    wo_sb[:, ct, mh * 512:(mh + 1) * 512],
                    start=(ct == 0),
                    stop=(ct == NPAIR - 2),
                )
            with nc.allow_low_precision(reason="bf16 partial out"):
                nc.vector.tensor_copy(osb[:, mh * 512:(mh + 1) * 512], ops[:])

        def oproj_b(tb):
            col = (NQC - 1) * 512 + tb * P
            osb = last_osb[tb]
            ops = ps_s.tile([P, 2, 512], f32, tag="s", name="ops_b")
            for mh in range(2):
                nc.tensor.matmul(
                    ops[:, mh, :],
                    ynorm[:, NPAIR - 1, col:col + P],
                    wo_sb[:, NPAIR - 1, mh * 512:(mh + 1) * 512],
                    start=True,
                    stop=True,
                )
            with nc.allow_low_precision(reason="bf16 partial out"):
                nc.vector.tensor_tensor(
                    osb[:].rearrange("p (m n) -> p m n", m=2),
                    osb[:].rearrange("p (m n) -> p m n", m=2),
                    ops[:], mybir.AluOpType.add)
            nc.sync.dma_start(out_d.ap()[col:col + P, :], osb[:])

        # Wave 0: only pair 0's projections + the first V block run up
        # front; V(1..3) streams in as filler one key-block ahead of
        # its use, and pair p+1's projections are emitted right after
        # attn(p, 0), so the PE starts as soon as ~1.2MB of DMA landed
        # (the whole early region is DMA-paced anyway).
        qk_chain("q", 0, 0)
        qk_chain("k", 0, 0)
        v_chain(0)
        for kb in range(1, 4):
            fillers.append(lambda kb=kb: v_chain(kb))

        for qc in range(NQC):
            if qc + 1 < NQC:
                # Pair 0's next-chunk projections must land before
                # attn(0, qc+1); pairs 1-3 of the LAST wave are pushed
                # into qc3 itself, which otherwise runs out of filler
                # and exposes the exp pipeline latency.
                late = NPAIR if qc + 2 < NQC else 1
                for p in range(late):
                    fillers.append(lambda p=p, t=qc + 1: qk_chain("q", p, t))
                    fillers.append(lambda p=p, t=qc + 1: qk_chain("k", p, t))
                for kb in range(4 * (qc + 1), 4 * (qc + 2)):
                    fillers.append(lambda kb=kb: v_chain(kb))
            if qc == NQC - 1:
                for p in range(1, NPAIR):
                    fillers.append(lambda p=p, t=qc: qk_chain("q", p, t))
                    fillers.append(lambda p=p, t=qc: qk_chain("k", p, t))
            if qc > 0:
                for tb in range(4):
                    fillers.append(lambda q=qc - 1, tb=tb: oproj_tb(q, tb))
            for p in range(NPAIR):
                attn(p, qc)
                if qc == 0 and p + 1 < NPAIR:
                    qk_chain("q", p + 1, 0)
                    qk_chain("k", p + 1, 0)
                if qc == NQC - 1 and p == NPAIR - 2:
                    for tb in range(4):
                        for mh in range(2):
                            fillers.append(
                                lambda tb=tb, mh=mh: oproj_a(tb, mh))

        while fillers:
            fillers.pop(0)()
        for tb in range(4):
            oproj_b(tb)


_NC_CACHE = {}


def _get_nc():
    if "nc" not in _NC_CACHE:
        _NC_CACHE["nc"] = build_kernel()
    return _NC_CACHE["nc"]


def kernel(x, Wq, bq, Wk, bk, Wv, bv, Wo, bo):
    x = np.asarray(x, dtype=np.float32)
    Wq = np.asarray(Wq, dtype=np.float32)
    Wk = np.asarray(Wk, dtype=np.float32)
    Wv = np.asarray(Wv, dtype=np.float32)
    Wo = np.asarray(Wo, dtype=np.float32)
    bf = ml_dtypes.bfloat16

    # Weight layouts (all contiguous per SBUF partition):
    #   wq/wk[m]: [pair, p, ct, n]  (pair's 128 W-rows transposed)
    #   wv[m]:    [p, ct, 512]
    #   wo[m]:    [p, ct(=pair), 1024]
    Wqp = Wq.reshape(2 * NPAIR, P, CT, P)          # [gpair, n, ct, p]
    Wkp = Wk.reshape(2 * NPAIR, P, CT, P)
    wq_r = [
        np.ascontiguousarray(
            Wqp[NPAIR * m:NPAIR * (m + 1)].transpose(0, 3, 2, 1)).astype(bf)
        for m in range(2)
    ]
    wk_r = [
        np.ascontiguousarray(
            Wkp[NPAIR * m:NPAIR * (m + 1)].transpose(0, 3, 2, 1)).astype(bf)
        for m in range(2)
    ]
    Wvp = Wv.reshape(2, 512, CT, P)                # [m, n, ct, p]
    wv_r = [
        np.ascontiguousarray(Wvp[m].transpose(2, 1, 0)).astype(bf)
        for m in range(2)
    ]
    # wo: contraction rows = my 512 y-dims -> [p, ct, n]:
    # wo_r[m][p, ct, n] = Wo[n, 512m + ct*128 + p]
    Wop = Wo.T.reshape(2, NPAIR, P, D)             # [m, ct, p, n]
    wo_r = [
        np.ascontiguousarray(Wop[m].transpose(1, 0, 2)).astype(bf)
        for m in range(2)
    ]
    tri = (np.arange(P)[:, None] <= np.arange(P)[None, :]).astype(bf)
    # osel[*, hh, c] = 1 where head hh of a pair owns partition c.
    osel = np.zeros((P, 2, P), dtype=np.float32)
    osel[:, 0, 0:DH] = 1.0
    osel[:, 1, DH:P] = 1.0
    osel = osel.astype(bf)

    # x: [tc, p, ct, 512] with x[b].T[ct*128+p, tc*512+j]
    xT_b = [
        np.ascontiguousarray(
            x[b].T.reshape(CT, P, NQC, 512).transpose(2, 1, 0, 3)
        ).astype(bf)
        for b in range(B)
    ]
    in_maps = []
    for c in range(NCORES):
        b, m = c // 2, c % 2
        in_maps.append({
            "xT": xT_b[b],
            "wq": wq_r[m],
            "wk": wk_r[m],
            "wv": wv_r[m],
            "wo": wo_r[m],
            "mask": tri,
            "osel": osel,
        })

    global _last_in_maps
    _last_in_maps = in_maps
    nc = _get_nc()
    res = run_bass_kernel_spmd(nc, in_maps, core_ids=list(range(NCORES)))

    out = np.empty((B, T, D), dtype=np.float32)
    for b in range(B):
        out[b] = (res.results[2 * b]["out"].astype(np.float32)
                  + res.results[2 * b + 1]["out"].astype(np.float32))
    return out


# revision 42
# speedup vs baseline: 1.0185x; 1.0185x over previous
"""Causal self-attention on 8 trn2 NeuronCores.

Problem: B=4, T=2048, D=1024, H=16 heads (Dh=64), fp32 in/out, causal
softmax attention with 4 linear projections (biases are zero in this
problem's setup and are folded out).

Sharding (SPMD, one NEFF on all 8 cores, no collectives):
  core c -> batch b = c//2, head-group m = c%2 (8 heads each).
  Each core computes Q/K/V for its 8 heads over ALL 2048 tokens of its
  batch, runs causal attention, and produces a PARTIAL output
  projection (contracting only its 512 y-dims of Wo).  The host sums
  the two partial outputs per batch (Megatron row-parallel unshard).
  This halves the K/V projection work vs. a query-split sharding (no
  duplicated K/V) and halves the per-core output projection.

Device pipeline per core (all matmul inputs bf16, fp32 PSUM):
  - Q/K projections per head-pair into transposed layout qT/kT [d, t]
    (so scores S^T = kT.T @ qT need no on-device transposes).
  - V projection per key-block into natural layout [t, d] for all 8
    heads at once, stored bf16 with an appended ones column (the ones
    accumulate the softmax denominator during the exp(S) @ V matmul).
  - Attention runs per 512-column query chunk qc (kb <= 4*qc+3 by
    causality): both heads of a pair land in one 2-bank PSUM tile, one
    exp op (ScalarE, 1/8 scale folded in) covers both heads, a single
    128x128 triangular mask multiplies the diagonal block on GpSimd,
    exp(S) @ V accumulates per-chunk Y tiles [65, 512] in PSUM.
  - Per-pair normalize with no DRAM round trip: the two denominator
    rows are staged to SBUF, two K=1 PE matmuls broadcast them across
    the pair's 128 partitions into one PSUM bank, one DVE reciprocal
    + multiply normalizes the pair's y chunk in place.
  - The partial output projection + DMA-out run per query chunk,
    interleaved into the next chunk's attention as PE filler; its
    per-ct matmuls are data-gated on each pair's normalize, so the
    kernel tail is only the last pair's normalize + 8 tail matmuls.
  - Host-side input layouts are arranged so every big DMA reads
    contiguous >=2KB per-partition segments.
"""

import numpy as np
import ml_dtypes

import concourse.bass as bass
import concourse.mybir as mybir
import concourse.tile as tile
from concourse import bacc
from concourse.bass_utils import run_bass_kernel_spmd

B, T, D, H, DH = 4, 2048, 1024, 16, 64
P = 128
CT = D // P          # 8 contraction tiles over the model dim
NKB = T // P         # 16 key blocks
NQC = T // 512       # 4 query chunks of 512
NPAIR = 4            # local head pairs per core (8 heads)
NCORES = 8

f32 = mybir.dt.float32
bf16 = mybir.dt.bfloat16
AF = mybir.ActivationFunctionType


def build_kernel():
    nc = bacc.Bacc("TRN2", target_bir_lowering=False, debug=False)
    xT_d = nc.dram_tensor("xT", [NQC, P, CT, 512], bf16, kind="ExternalInput")
    wq_d = nc.dram_tensor("wq", [NPAIR, P, CT, P], bf16, kind="ExternalInput")
    wk_d = nc.dram_tensor("wk", [NPAIR, P, CT, P], bf16, kind="ExternalInput")
    wv_d = nc.dram_tensor("wv", [P, CT, 512], bf16, kind="ExternalInput")
    wo_d = nc.dram_tensor("wo", [P, NPAIR, D], bf16, kind="ExternalInput")
    mask_d = nc.dram_tensor("mask", [P, P], bf16, kind="ExternalInput")
    osel_d = nc.dram_tensor("osel", [P, 2, P], bf16, kind="ExternalInput")
    out_d = nc.dram_tensor("out", [T, D], bf16, kind="ExternalOutput")
    with tile.TileContext(nc) as tc:
        _emit(tc, xT_d, wq_d, wk_d, wv_d, wo_d, mask_d, osel_d, out_d)
    nc.compile()
    return nc


def _emit(tc, xT_d, wq_d, wk_d, wv_d, wo_d, mask_d, osel_d, out_d):
    nc = tc.nc
    with (
        tc.tile_pool(name="xt", bufs=1) as xt_pool,
        tc.tile_pool(name="w", bufs=1) as w_pool,
        tc.tile_pool(name="qkv", bufs=1) as qkv_pool,
        tc.tile_pool(name="ynorm", bufs=1) as ynorm_pool,
        tc.tile_pool(name="exp", bufs=6) as exp_pool,
        tc.tile_pool(name="den", bufs=2) as den_pool,
        tc.tile_pool(name="norm", bufs=2) as norm_pool,
        tc.tile_pool(name="osb", bufs=2) as osb_pool,
        tc.tile_pool(name="osbl", bufs=1) as osbl_pool,
        tc.tile_pool(name="ps_s", bufs=2, space="PSUM") as ps_s,
        tc.tile_pool(name="ps_y", bufs=2, space="PSUM") as ps_y,
        tc.tile_pool(name="ps_p", bufs=2, space="PSUM") as ps_p,
    ):
        xt = xt_pool.tile([P, NQC, CT, 512], bf16, name="xt")
        wq_sb = w_pool.tile([P, NPAIR, CT, P], bf16, name="wq")
        wk_sb = w_pool.tile([P, NPAIR, CT, P], bf16, name="wk")
        wv_sb = w_pool.tile([P, CT, 512], bf16, name="wv")
        wo_sb = w_pool.tile([P, NPAIR, D], bf16, name="wo")
        mask_sb = w_pool.tile([P, P], bf16, name="mask")
        osel_sb = w_pool.tile([P, 2, P], bf16, name="osel")
        qT = qkv_pool.tile([P, NPAIR, T], bf16, name="qT")
        kT = qkv_pool.tile([P, NPAIR, T], bf16, name="kT")
        v8 = qkv_pool.tile([P, NKB, 8, DH + 1], bf16, name="v8")
        ynorm = ynorm_pool.tile([P, NPAIR, T], bf16, name="ynorm")
        nc.vector.memset(v8[:, :, :, DH:DH + 1], 1.0)

        # Input DMAs, ordered so the first Q/K chains (pair 0, token
        # chunk 0) can start DMA-paced as early as possible.  All
        # sources are host-pre-arranged to be contiguous per partition;
        # the first chunk's x / wv transfers are split by ct-half so
        # the projection chains stream behind the DMA.
        HC = CT // 2
        nc.sync.dma_start(wq_sb[:, 0, :, :], wq_d.ap()[0])
        nc.sync.dma_start(xt[:, 0, 0:HC, :], xT_d.ap()[0][:, 0:HC, :])
        nc.sync.dma_start(wk_sb[:, 0, :, :], wk_d.ap()[0])
        nc.sync.dma_start(xt[:, 0, HC:CT, :], xT_d.ap()[0][:, HC:CT, :])
        nc.sync.dma_start(wv_sb[:, 0:HC, :], wv_d.ap()[:, 0:HC, :])
        nc.sync.dma_start(wv_sb[:, HC:CT, :], wv_d.ap()[:, HC:CT, :])
        for p in range(1, NPAIR):
            nc.sync.dma_start(wq_sb[:, p, :, :], wq_d.ap()[p])
            nc.sync.dma_start(wk_sb[:, p, :, :], wk_d.ap()[p])
        nc.sync.dma_start(mask_sb[:], mask_d.ap())
        nc.sync.dma_start(osel_sb[:], osel_d.ap())
        for tcc in range(1, NQC):
            nc.sync.dma_start(xt[:, tcc, :, :], xT_d.ap()[tcc])
        nc.sync.dma_start(wo_sb[:], wo_d.ap())

        def qk_chain(which, p, tcc):
            w_sb, dstT = (wq_sb, qT) if which == "q" else (wk_sb, kT)
            ps = ps_p.tile([P, 512], f32, tag="pp", name="ps_qk")
            for ct in range(CT):
                nc.tensor.matmul(
                    ps[:],
                    w_sb[:, p, ct, :],
                    xt[:, tcc, ct, :],
                    start=(ct == 0),
                    stop=(ct == CT - 1),
                )
            nc.vector.tensor_copy(
                dstT[:, p, tcc * 512:(tcc + 1) * 512], ps[:])

        def v_chain(kb):
            ps = ps_p.tile([P, 512], f32, tag="pp", name="ps_v")
            for ct in range(CT):
                nc.tensor.matmul(
                    ps[:],
                    xt[:, kb // 4, ct, (kb % 4) * P:(kb % 4 + 1) * P],
                    wv_sb[:, ct, :],
                    start=(ct == 0),
                    stop=(ct == CT - 1),
                )
            nc.vector.tensor_copy(
                v8[:, kb, :, 0:DH], ps[:].rearrange("p (h d) -> p h d", h=8)
            )

        fillers = []

        def tick():
            if fillers:
                fillers.pop(0)()

        def attn(p, qc):
            last = 4 * qc + 3
            sl = slice(qc * 512, (qc + 1) * 512)
            ys = [
                ps_y.tile([DH + 1, 512], f32, tag="y", name=f"y{hh}")
                for hh in range(2)
            ]
            for kb in range(last + 1):
                diag = kb >= 4 * qc
                s0 = kb * P if diag else qc * 512
                w = (qc + 1) * 512 - s0
                sc = ps_s.tile([P, 2, 512], f32, tag="s", name="sc")
                for hh in range(2):
                    nc.tensor.matmul(
                        sc[:, hh, 0:w],
                        kT[hh * DH:(hh + 1) * DH, p, kb * P:(kb + 1) * P],
                        qT[hh * DH:(hh + 1) * DH, p, s0:s0 + w],
                        start=True,
                        stop=True,
                    )
                expS = exp_pool.tile([P, 2, 512], bf16, tag="e", name="expS")
                nc.scalar.activation(
                    expS[:, :, 0:w], sc[:, :, 0:w], AF.Exp, scale=0.125
                )
                if diag:
                    for hh in range(2):
                        nc.vector.tensor_mul(
                            expS[:, hh, 0:P], expS[:, hh, 0:P], mask_sb[:]
                        )
                for hh in range(2):
                    nc.tensor.matmul(
                        ys[hh][:, s0 - qc * 512:s0 - qc * 512 + w],
                        v8[:, kb, 2 * p + hh, :],
                        expS[:, hh, 0:w],
                        start=(kb == 0),
                        stop=(kb == last),
                    )
                tick()
            # Drain: stage denominator rows and the pair's y chunk; the
            # normalize (two K=1 broadcast matmuls + reciprocal +
            # multiply) is deferred into the filler stream so the PE
            # never waits on the staging copies.
            dstg = den_pool.tile([DH + 1, 2, 512], bf16, tag="dstg",
                                 name="dstg")
            with nc.allow_low_precision(reason="bf16 denominator staging"):
                for hh in range(2):
                    nc.vector.tensor_copy(dstg[DH:DH + 1, hh, :],
                                          ys[hh][DH:DH + 1, :])
            for hh in range(2):
                nc.vector.tensor_copy(
                    ynorm[hh * DH:(hh + 1) * DH, p, sl], ys[hh][0:DH, :])

            def norm_pair(p=p, sl=sl, dstg=dstg):
                bc_ps = ps_p.tile([P, 512], f32, tag="pp", name="bc_ps")
                for hh in range(2):
                    nc.tensor.matmul(
                        bc_ps[:],
                        osel_sb[DH:DH + 1, hh, :],
                        dstg[DH:DH + 1, hh, :],
                        start=(hh == 0),
                        stop=(hh == 1),
                    )
                rec = norm_pool.tile([P, 512], f32, tag="rec", name="rec")
                nc.vector.reciprocal_approx_fast(rec[:], bc_ps[:])
                rec_h = norm_pool.tile([P, 512], bf16, tag="rech",
                                       name="rec_h")
                with nc.allow_low_precision(reason="bf16 recip"):
                    nc.vector.tensor_copy(rec_h[:], rec[:])
                nc.vector.tensor_mul(ynorm[:, p, sl], ynorm[:, p, sl],
                                     rec_h[:])

            fillers.append(norm_pair)

        def oproj_tb(qc, tb):
            col = qc * 512 + tb * P
            osb = osb_pool.tile([P, D], bf16, tag="osb", name="osb")
            for mh in range(2):
                ops = ps_p.tile([P, 512], f32, tag="pp", name="ops")
                for ct in range(NPAIR):
                    nc.tensor.matmul(
                        ops[:],
                        ynorm[:, ct, col:col + P],
                        wo_sb[:, ct, mh * 512:(mh + 1) * 512],
                        start=(ct == 0),
                        stop=(ct == NPAIR - 1),
                    )
                with nc.allow_low_precision(reason="bf16 partial out"):
                    nc.vector.tensor_copy(osb[:, mh * 512:(mh + 1) * 512],
                                          ops[:])
            nc.sync.dma_start(out_d.ap()[col:col + P, :], osb[:])

        # Tail-split output projection for the last chunk: the ct<3
        # partial contraction runs as filler during the last pair's
        # attention; only the 8 ct=3 matmuls + adds trail the final
        # normalize.
        last_osb = {}

        def oproj_a(tb, mh):
            col = (NQC - 1) * 512 + tb * P
            if tb not in last_osb:
                last_osb[tb] = osbl_pool.tile([P, D], bf16, name=f"osbl{tb}")
            osb = last_osb[tb]
            ops = ps_p.tile([P, 512], f32, tag="pp", name="ops_a")
            for ct in range(NPAIR - 1):
                nc.tensor.matmul(
                    ops[:],
                    ynorm[:, ct, col:col + P],
                    wo_sb[:, ct, mh * 512:(mh + 1) * 512],
                    start=(ct == 0),
                    stop=(ct == NPAIR - 2),
                )
            with nc.allow_low_precision(reason="bf16 partial out"):
                nc.vector.tensor_copy(osb[:, mh * 512:(mh + 1) * 512], ops[:])

        def oproj_b(tb):
            col = (NQC - 1) * 512 + tb * P
            osb = last_osb[tb]
            ops = ps_s.tile([P, 2, 512], f32, tag="s", name="ops_b")
            for mh in range(2):
                nc.tensor.matmul(
                    ops[:, mh, :],
                    ynorm[:, NPAIR - 1, col:col + P],
                    wo_sb[:, NPAIR - 1, mh * 512:(mh + 1) * 512],
                    start=True,
                    stop=True,
                )
            with nc.allow_low_precision(reason="bf16 partial out"):
                nc.vector.tensor_tensor(
                    osb[:].rearrange("p (m n) -> p m n", m=2),
                    osb[:].rearrange("p (m n) -> p m n", m=2),
                    ops[:], mybir.AluOpType.add)
            nc.sync.dma_start(out_d.ap()[col:col + P, :], osb[:])

        # Wave 0: only pair 0's projections + the first V block run up
        # front; V(1..3) streams in as filler one key-block ahead of
        # its use, and pair p+1's projections are emitted right after
        # attn(p, 0), so the PE starts as soon as ~1.2MB of DMA landed
        # (the whole early region is DMA-paced anyway).
        qk_chain("q", 0, 0)
        qk_chain("k", 0, 0)
        v_chain(0)
        for kb in range(1, 4):
            fillers.append(lambda kb=kb: v_chain(kb))

        for qc in range(NQC):
            if qc + 1 < NQC:
                for p in range(NPAIR):
                    fillers.append(lambda p=p, t=qc + 1: qk_chain("q", p, t))
                    fillers.append(lambda p=p, t=qc + 1: qk_chain("k", p, t))
                for kb in range(4 * (qc + 1), 4 * (qc + 2)):
                    fillers.append(lambda kb=kb: v_chain(kb))
            if qc > 0:
                for tb in range(4):
                    fillers.append(lambda q=qc - 1, tb=tb: oproj_tb(q, tb))
            for p in range(NPAIR):
                attn(p, qc)
                if qc == 0 and p + 1 < NPAIR:
                    qk_chain("q", p + 1, 0)
                    qk_chain("k", p + 1, 0)
                if qc == NQC - 1 and p == NPAIR - 2:
                    for tb in range(4):
                        for mh in range(2):
                            fillers.append(
                                lambda tb=tb, mh=mh: oproj_a(tb, mh))

        while fillers:
            fillers.pop(0)()
        for tb in range(4):
            oproj_b(tb)


_NC_CACHE = {}


def _get_nc():
    if "nc" not in _NC_CACHE:
        _NC_CACHE["nc"] = build_kernel()
    return _NC_CACHE["nc"]


def kernel(x, Wq, bq, Wk, bk, Wv, bv, Wo, bo):
    x = np.asarray(x, dtype=np.float32)
    Wq = np.asarray(Wq, dtype=np.float32)
    Wk = np.asarray(Wk, dtype=np.float32)
    Wv = np.asarray(Wv, dtype=np.float32)
    Wo = np.asarray(Wo, dtype=np.float32)
    bf = ml_dtypes.bfloat16

    # Weight layouts (all contiguous per SBUF partition):
    #   wq/wk[m]: [pair, p, ct, n]  (pair's 128 W-rows transposed)
    #   wv[m]:    [p, ct, 512]
    #   wo[m]:    [p, ct(=pair), 1024]
    Wqp = Wq.reshape(2 * NPAIR, P, CT, P)          # [gpair, n, ct, p]
    Wkp = Wk.reshape(2 * NPAIR, P, CT, P)
    wq_r = [
        np.ascontiguousarray(
            Wqp[NPAIR * m:NPAIR * (m + 1)].transpose(0, 3, 2, 1)).astype(bf)
        for m in range(2)
    ]
    wk_r = [
        np.ascontiguousarray(
            Wkp[NPAIR * m:NPAIR * (m + 1)].transpose(0, 3, 2, 1)).astype(bf)
        for m in range(2)
    ]
    Wvp = Wv.reshape(2, 512, CT, P)                # [m, n, ct, p]
    wv_r = [
        np.ascontiguousarray(Wvp[m].transpose(2, 1, 0)).astype(bf)
        for m in range(2)
    ]
    # wo: contraction rows = my 512 y-dims -> [p, ct, n]:
    # wo_r[m][p, ct, n] = Wo[n, 512m + ct*128 + p]
    Wop = Wo.T.reshape(2, NPAIR, P, D)             # [m, ct, p, n]
    wo_r = [
        np.ascontiguousarray(Wop[m].transpose(1, 0, 2)).astype(bf)
        for m in range(2)
    ]
    tri = (np.arange(P)[:, None] <= np.arange(P)[None, :]).astype(bf)
    # osel[*, hh, c] = 1 where head hh of a pair owns partition c.
    osel = np.zeros((P, 2, P), dtype=np.float32)
    osel[:, 0, 0:DH] = 1.0
    osel[:, 1, DH:P] = 1.0
    osel = osel.astype(bf)

    # x: [tc, p, ct, 512] with x[b].T[ct*128+p, tc*512+j]
    xT_b = [
        np.ascontiguousarray(
            x[b].T.reshape(CT, P, NQC, 512).transpose(2, 1, 0, 3)
        ).astype(bf)
        for b in range(B)
    ]
    in_maps = []
    for c in range(NCORES):
        b, m = c // 2, c % 2
        in_maps.append({
            "xT": xT_b[b],
            "wq": wq_r[m],
            "wk": wk_r[m],
            "wv": wv_r[m],
            "wo": wo_r[m],
            "mask": tri,
            "osel": osel,
        })

    global _last_in_maps
    _last_in_maps = in_maps
    nc = _get_nc()
    res = run_bass_kernel_spmd(nc, in_maps, core_ids=list(range(NCORES)))

    out = np.empty((B, T, D), dtype=np.float32)
    for b in range(B):
        out[b] = (res.results[2 * b]["out"].astype(np.float32)
                  + res.results[2 * b + 1]["out"].astype(np.float32))
    return out


# revision 44
# speedup vs baseline: 1.0301x; 1.0114x over previous
"""Causal self-attention on 8 trn2 NeuronCores.

Problem: B=4, T=2048, D=1024, H=16 heads (Dh=64), fp32 in/out, causal
softmax attention with 4 linear projections (biases are zero in this
problem's setup and are folded out).

Sharding (SPMD, one NEFF on all 8 cores, no collectives):
  core c -> batch b = c//2, head-group m = c%2 (8 heads each).
  Each core computes Q/K/V for its 8 heads over ALL 2048 tokens of its
  batch, runs causal attention, and produces a PARTIAL output
  projection (contracting only its 512 y-dims of Wo).  The host sums
  the two partial outputs per batch (Megatron row-parallel unshard).
  This halves the K/V projection work vs. a query-split sharding (no
  duplicated K/V) and halves the per-core output projection.

Device pipeline per core (all matmul inputs bf16, fp32 PSUM):
  - Q/K projections per head-pair into transposed layout qT/kT [d, t]
    (so scores S^T = kT.T @ qT need no on-device transposes).
  - V projection per key-block into natural layout [t, d] for all 8
    heads at once, stored bf16 with an appended ones column (the ones
    accumulate the softmax denominator during the exp(S) @ V matmul).
  - Attention runs per 512-column query chunk qc (kb <= 4*qc+3 by
    causality): both heads of a pair land in one 2-bank PSUM tile, one
    exp op (ScalarE, 1/8 scale folded in) covers both heads, a single
    128x128 triangular mask multiplies the diagonal block on GpSimd,
    exp(S) @ V accumulates per-chunk Y tiles [65, 512] in PSUM.
  - Per-pair normalize with no DRAM round trip: the two denominator
    rows are staged to SBUF, two K=1 PE matmuls broadcast them across
    the pair's 128 partitions into one PSUM bank, one DVE reciprocal
    + multiply normalizes the pair's y chunk in place.
  - The partial output projection + DMA-out run per query chunk,
    interleaved into the next chunk's attention as PE filler; its
    per-ct matmuls are data-gated on each pair's normalize, so the
    kernel tail is only the last pair's normalize + 8 tail matmuls.
  - Host-side input layouts are arranged so every big DMA reads
    contiguous >=2KB per-partition segments.
"""

import numpy as np
import ml_dtypes

import concourse.bass as bass
import concourse.mybir as mybir
import concourse.tile as tile
from concourse import bacc
from concourse.bass_utils import run_bass_kernel_spmd

B, T, D, H, DH = 4, 2048, 1024, 16, 64
P = 128
CT = D // P          # 8 contraction tiles over the model dim
NKB = T // P         # 16 key blocks
NQC = T // 512       # 4 query chunks of 512
NPAIR = 4            # local head pairs per core (8 heads)
NCORES = 8

f32 = mybir.dt.float32
bf16 = mybir.dt.bfloat16
AF = mybir.ActivationFunctionType


def build_kernel():
    nc = bacc.Bacc("TRN2", target_bir_lowering=False, debug=False)
    xT_d = nc.dram_tensor("xT", [NQC, P, CT, 512], bf16, kind="ExternalInput")
    wq_d = nc.dram_tensor("wq", [NPAIR, P, CT, P], bf16, kind="ExternalInput")
    wk_d = nc.dram_tensor("wk", [NPAIR, P, CT, P], bf16, kind="ExternalInput")
    wv_d = nc.dram_tensor("wv", [P, CT, 512], bf16, kind="ExternalInput")
    wo_d = nc.dram_tensor("wo", [P, NPAIR, D], bf16, kind="ExternalInput")
    mask_d = nc.dram_tensor("mask", [P, P], bf16, kind="ExternalInput")
    osel_d = nc.dram_tensor("osel", [P, 2, P], bf16, kind="ExternalInput")
    out_d = nc.dram_tensor("out", [T, D], bf16, kind="ExternalOutput")
    with tile.TileContext(nc) as tc:
        _emit(tc, xT_d, wq_d, wk_d, wv_d, wo_d, mask_d, osel_d, out_d)
    nc.compile()
    return nc


def _emit(tc, xT_d, wq_d, wk_d, wv_d, wo_d, mask_d, osel_d, out_d):
    nc = tc.nc
    with (
        tc.tile_pool(name="xt", bufs=1) as xt_pool,
        tc.tile_pool(name="w", bufs=1) as w_pool,
        tc.tile_pool(name="qkv", bufs=1) as qkv_pool,
        tc.tile_pool(name="ynorm", bufs=1) as ynorm_pool,
        tc.tile_pool(name="exp", bufs=6) as exp_pool,
        tc.tile_pool(name="den", bufs=2) as den_pool,
        tc.tile_pool(name="norm", bufs=2) as norm_pool,
        tc.tile_pool(name="osb", bufs=2) as osb_pool,
        tc.tile_pool(name="osbl", bufs=1) as osbl_pool,
        tc.tile_pool(name="ps_s", bufs=2, space="PSUM") as ps_s,
        tc.tile_pool(name="ps_y", bufs=2, space="PSUM") as ps_y,
        tc.tile_pool(name="ps_p", bufs=2, space="PSUM") as ps_p,
    ):
        xt = xt_pool.tile([P, NQC, CT, 512], bf16, name="xt")
        wq_sb = w_pool.tile([P, NPAIR, CT, P], bf16, name="wq")
        wk_sb = w_pool.tile([P, NPAIR, CT, P], bf16, name="wk")
        wv_sb = w_pool.tile([P, CT, 512], bf16, name="wv")
        wo_sb = w_pool.tile([P, NPAIR, D], bf16, name="wo")
        mask_sb = w_pool.tile([P, P], bf16, name="mask")
        osel_sb = w_pool.tile([P, 2, P], bf16, name="osel")
        qT = qkv_pool.tile([P, NPAIR, T], bf16, name="qT")
        kT = qkv_pool.tile([P, NPAIR, T], bf16, name="kT")
        v8 = qkv_pool.tile([P, NKB, 8, DH + 1], bf16, name="v8")
        ynorm = ynorm_pool.tile([P, NPAIR, T], bf16, name="ynorm")
        nc.vector.memset(v8[:, :, :, DH:DH + 1], 1.0)

        # Input DMAs, ordered so the first Q/K chains (pair 0, token
        # chunk 0) can start DMA-paced as early as possible.  All
        # sources are host-pre-arranged to be contiguous per partition;
        # the first chunk's x / wv transfers are split by ct-half so
        # the projection chains stream behind the DMA.
        HC = CT // 2
        nc.sync.dma_start(wq_sb[:, 0, :, :], wq_d.ap()[0])
        nc.sync.dma_start(xt[:, 0, 0:HC, :], xT_d.ap()[0][:, 0:HC, :])
        nc.sync.dma_start(wk_sb[:, 0, :, :], wk_d.ap()[0])
        nc.sync.dma_start(xt[:, 0, HC:CT, :], xT_d.ap()[0][:, HC:CT, :])
        nc.sync.dma_start(wv_sb[:, 0:HC, :], wv_d.ap()[:, 0:HC, :])
        nc.sync.dma_start(wv_sb[:, HC:CT, :], wv_d.ap()[:, HC:CT, :])
        for p in range(1, NPAIR):
            nc.sync.dma_start(wq_sb[:, p, :, :], wq_d.ap()[p])
            nc.sync.dma_start(wk_sb[:, p, :, :], wk_d.ap()[p])
        nc.sync.dma_start(mask_sb[:], mask_d.ap())
        nc.sync.dma_start(osel_sb[:], osel_d.ap())
        for tcc in range(1, NQC):
            nc.sync.dma_start(xt[:, tcc, :, :], xT_d.ap()[tcc])
        nc.sync.dma_start(wo_sb[:], wo_d.ap())

        def qk_chain(which, p, tcc):
            w_sb, dstT = (wq_sb, qT) if which == "q" else (wk_sb, kT)
            ps = ps_p.tile([P, 512], f32, tag="pp", name="ps_qk")
            for ct in range(CT):
                nc.tensor.matmul(
                    ps[:],
                    w_sb[:, p, ct, :],
                    xt[:, tcc, ct, :],
                    start=(ct == 0),
                    stop=(ct == CT - 1),
                )
            nc.vector.tensor_copy(
                dstT[:, p, tcc * 512:(tcc + 1) * 512], ps[:])

        def v_chain(kb):
            ps = ps_p.tile([P, 512], f32, tag="pp", name="ps_v")
            for ct in range(CT):
                nc.tensor.matmul(
                    ps[:],
                    xt[:, kb // 4, ct, (kb % 4) * P:(kb % 4 + 1) * P],
                    wv_sb[:, ct, :],
                    start=(ct == 0),
                    stop=(ct == CT - 1),
                )
            nc.vector.tensor_copy(
                v8[:, kb, :, 0:DH], ps[:].rearrange("p (h d) -> p h d", h=8)
            )

        fillers = []

        def tick():
            if fillers:
                fillers.pop(0)()

        def attn(p, qc):
            last = 4 * qc + 3
            sl = slice(qc * 512, (qc + 1) * 512)
            ys = [
                ps_y.tile([DH + 1, 512], f32, tag="y", name=f"y{hh}")
                for hh in range(2)
            ]
            for kb in range(last + 1):
                diag = kb >= 4 * qc
                s0 = kb * P if diag else qc * 512
                w = (qc + 1) * 512 - s0
                sc = ps_s.tile([P, 2, 512], f32, tag="s", name="sc")
                for hh in range(2):
                    nc.tensor.matmul(
                        sc[:, hh, 0:w],
                        kT[hh * DH:(hh + 1) * DH, p, kb * P:(kb + 1) * P],
                        qT[hh * DH:(hh + 1) * DH, p, s0:s0 + w],
                        start=True,
                        stop=True,
                    )
                expS = exp_pool.tile([P, 2, 512], bf16, tag="e", name="expS")
                nc.scalar.activation(
                    expS[:, :, 0:w], sc[:, :, 0:w], AF.Exp, scale=0.125
                )
                if diag:
                    for hh in range(2):
                        nc.vector.tensor_mul(
                            expS[:, hh, 0:P], expS[:, hh, 0:P], mask_sb[:]
                        )
                for hh in range(2):
                    nc.tensor.matmul(
                        ys[hh][:, s0 - qc * 512:s0 - qc * 512 + w],
                        v8[:, kb, 2 * p + hh, :],
                        expS[:, hh, 0:w],
                        start=(kb == 0),
                        stop=(kb == last),
                    )
                tick()
            # Drain: stage denominator rows and the pair's y chunk; the
            # normalize (two K=1 broadcast matmuls + reciprocal +
            # multiply) is deferred into the filler stream so the PE
            # never waits on the staging copies.
            dstg = den_pool.tile([DH + 1, 2, 512], bf16, tag="dstg",
                                 name="dstg")
            with nc.allow_low_precision(reason="bf16 denominator staging"):
                for hh in range(2):
                    nc.vector.tensor_copy(dstg[DH:DH + 1, hh, :],
                                          ys[hh][DH:DH + 1, :])
            for hh in range(2):
                nc.vector.tensor_copy(
                    ynorm[hh * DH:(hh + 1) * DH, p, sl], ys[hh][0:DH, :])

            def norm_pair(p=p, sl=sl, dstg=dstg):
                bc_ps = ps_p.tile([P, 512], f32, tag="pp", name="bc_ps")
                for hh in range(2):
                    nc.tensor.matmul(
                        bc_ps[:],
                        osel_sb[DH:DH + 1, hh, :],
                        dstg[DH:DH + 1, hh, :],
                        start=(hh == 0),
                        stop=(hh == 1),
                    )
                rec = norm_pool.tile([P, 512], f32, tag="rec", name="rec")
                nc.vector.reciprocal_approx_fast(rec[:], bc_ps[:])
                rec_h = norm_pool.tile([P, 512], bf16, tag="rech",
                                       name="rec_h")
                with nc.allow_low_precision(reason="bf16 recip"):
                    nc.vector.tensor_copy(rec_h[:], rec[:])
                nc.vector.tensor_mul(ynorm[:, p, sl], ynorm[:, p, sl],
                                     rec_h[:])

            fillers.append(norm_pair)

        def oproj_tb(qc, tb):
            col = qc * 512 + tb * P
            osb = osb_pool.tile([P, D], bf16, tag="osb", name="osb")
            for mh in range(2):
                ops = ps_p.tile([P, 512], f32, tag="pp", name="ops")
                for ct in range(NPAIR):
                    nc.tensor.matmul(
                        ops[:],
                        ynorm[:, ct, col:col + P],
                        wo_sb[:, ct, mh * 512:(mh + 1) * 512],
                        start=(ct == 0),
                        stop=(ct == NPAIR - 1),
                    )
                with nc.allow_low_precision(reason="bf16 partial out"):
                    nc.vector.tensor_copy(osb[:, mh * 512:(mh + 1) * 512],
                                          ops[:])
            nc.sync.dma_start(out_d.ap()[col:col + P, :], osb[:])

        # Tail-split output projection for the last chunk: the ct<3
        # partial contraction runs as filler during the last pair's
        # attention; only the 8 ct=3 matmuls + adds trail the final
        # normalize.
        last_osb = {}

        def oproj_a(tb, mh):
            col = (NQC - 1) * 512 + tb * P
            if tb not in last_osb:
                last_osb[tb] = osbl_pool.tile([P, D], bf16, name=f"osbl{tb}")
            osb = last_osb[tb]
            ops = ps_p.tile([P, 512], f32, tag="pp", name="ops_a")
            for ct in range(NPAIR - 1):
                nc.tensor.matmul(
                    ops[:],
                    ynorm[:, ct, col:col + P],
                    wo_sb[:, ct, mh * 512:(mh + 1) * 512],
                    start=(ct == 0),
                    stop=(ct == NPAIR - 2),
                )
            with nc.allow_low_precision(reason="bf16 partial out"):
                nc.vector.tensor_copy(osb[:, mh * 512:(mh + 1) * 512], ops[:])

        def oproj_b(tb, mh):
            col = (NQC - 1) * 512 + tb * P
            osb = last_osb[tb]
            ops = ps_p.tile([P, 512], f32, tag="pp", name="ops_b")
            nc.tensor.matmul(
                ops[:],
                ynorm[:, NPAIR - 1, col:col + P],
                wo_sb[:, NPAIR - 1, mh * 512:(mh + 1) * 512],
                start=True,
                stop=True,
            )
            sl = slice(mh * 512, (mh + 1) * 512)
            with nc.allow_low_precision(reason="bf16 partial out"):
                nc.vector.tensor_tensor(
                    osb[:, sl], osb[:, sl], ops[:], mybir.AluOpType.add)
            nc.sync.dma_start(out_d.ap()[col:col + P, sl], osb[:, sl])

        # Wave 0: only pair 0's projections + the first V block run up
        # front; V(1..3) streams in as filler one key-block ahead of
        # its use, and pair p+1's projections are emitted right after
        # attn(p, 0), so the PE starts as soon as ~1.2MB of DMA landed
        # (the whole early region is DMA-paced anyway).
        qk_chain("q", 0, 0)
        qk_chain("k", 0, 0)
        v_chain(0)
        for kb in range(1, 4):
            fillers.append(lambda kb=kb: v_chain(kb))

        for qc in range(NQC):
            if qc + 1 < NQC:
                for p in range(NPAIR):
                    fillers.append(lambda p=p, t=qc + 1: qk_chain("q", p, t))
                    fillers.append(lambda p=p, t=qc + 1: qk_chain("k", p, t))
                for kb in range(4 * (qc + 1), 4 * (qc + 2)):
                    fillers.append(lambda kb=kb: v_chain(kb))
            if qc > 0:
                for tb in range(4):
                    fillers.append(lambda q=qc - 1, tb=tb: oproj_tb(q, tb))
            for p in range(NPAIR):
                attn(p, qc)
                if qc == 0 and p + 1 < NPAIR:
                    qk_chain("q", p + 1, 0)
                    qk_chain("k", p + 1, 0)
                if qc == NQC - 1 and p == NPAIR - 2:
                    for tb in range(4):
                        for mh in range(2):
                            fillers.append(
                                lambda tb=tb, mh=mh: oproj_a(tb, mh))

        while fillers:
            fillers.pop(0)()
        for tb in range(4):
            for mh in range(2):
                oproj_b(tb, mh)


_NC_CACHE = {}


def _get_nc():
    if "nc" not in _NC_CACHE:
        _NC_CACHE["nc"] = build_kernel()
    return _NC_CACHE["nc"]


def kernel(x, Wq, bq, Wk, bk, Wv, bv, Wo, bo):
    x = np.asarray(x, dtype=np.float32)
    Wq = np.asarray(Wq, dtype=np.float32)
    Wk = np.asarray(Wk, dtype=np.float32)
    Wv = np.asarray(Wv, dtype=np.float32)
    Wo = np.asarray(Wo, dtype=np.float32)
    bf = ml_dtypes.bfloat16

    # Weight layouts (all contiguous per SBUF partition):
    #   wq/wk[m]: [pair, p, ct, n]  (pair's 128 W-rows transposed)
    #   wv[m]:    [p, ct, 512]
    #   wo[m]:    [p, ct(=pair), 1024]
    Wqp = Wq.reshape(2 * NPAIR, P, CT, P)          # [gpair, n, ct, p]
    Wkp = Wk.reshape(2 * NPAIR, P, CT, P)
    wq_r = [
        np.ascontiguousarray(
            Wqp[NPAIR * m:NPAIR * (m + 1)].transpose(0, 3, 2, 1)).astype(bf)
        for m in range(2)
    ]
    wk_r = [
        np.ascontiguousarray(
            Wkp[NPAIR * m:NPAIR * (m + 1)].transpose(0, 3, 2, 1)).astype(bf)
        for m in range(2)
    ]
    Wvp = Wv.reshape(2, 512, CT, P)                # [m, n, ct, p]
    wv_r = [
        np.ascontiguousarray(Wvp[m].transpose(2, 1, 0)).astype(bf)
        for m in range(2)
    ]
    # wo: contraction rows = my 512 y-dims -> [p, ct, n]:
    # wo_r[m][p, ct, n] = Wo[n, 512m + ct*128 + p]
    Wop = Wo.T.reshape(2, NPAIR, P, D)             # [m, ct, p, n]
    wo_r = [
        np.ascontiguousarray(Wop[m].transpose(1, 0, 2)).astype(bf)
        for m in range(2)
    ]
    tri = (np.arange(P)[:, None] <= np.arange(P)[None, :]).astype(bf)
    # osel[*, hh, c] = 1 where head hh of a pair owns partition c.
    osel = np.zeros((P, 2, P), dtype=np.float32)
    osel[:, 0, 0:DH] = 1.0
    osel[:, 1, DH:P] = 1.0
    osel = osel.astype(bf)

    # x: [tc, p, ct, 512] with x[b].T[ct*128+p, tc*512+j]
    xT_b = [
        np.ascontiguousarray(
            x[b].T.reshape(CT, P, NQC, 512).transpose(2, 1, 0, 3)
        ).astype(bf)
        for b in range(B)
    ]
    in_maps = []
    for c in range(NCORES):
        b, m = c // 2, c % 2
        in_maps.append({
            "xT": xT_b[b],
            "wq": wq_r[m],
            "wk": wk_r[m],
            "wv": wv_r[m],
            "wo": wo_r[m],
            "mask": tri,
            "osel": osel,
        })

    global _last_in_maps
    _last_in_maps = in_maps
    nc = _get_nc()
    res = run_bass_kernel_spmd(nc, in_maps, core_ids=list(range(NCORES)))

    out = np.empty((B, T, D), dtype=np.float32)
    for b in range(B):
        out[b] = (res.results[2 * b]["out"].astype(np.float32)
                  + res.results[2 * b + 1]["out"].astype(np.float32))
    return out
